# revision 22
# baseline (speedup 1.0000x reference)
"""Bass/Trainium2 kernel for nn_BailingMoELinearAttention.

Tensor-parallel over heads across 8 NeuronCores: each core owns 2 of the 16
heads (columns of Wqkv/Wg, rows of Wo). Per-core pipeline, fused per 128-token
chunk:
  qkvg projection (bf16 matmul) -> silu(q,k,v) / sigmoid(gate) -> per-head
  RMSNorm -> RoPE (host-precomputed cos/sin tables) -> chunked causal linear
  attention (running k^T v state, one PSUM bank per head) -> pre-gated
  activations g2 = o * g_norm_w * sigmoid(gate) and partial sum-of-squares.
Cross-core: AllReduce of per-token sum-of-squares (the group norm spans all 16
heads), per-token rstd scaling, then an AllToAll that exchanges the bf16 gated
activations so each core holds all 2048 inner columns for its 1024-token row
block, and a local out-projection against the full Wo. Host concatenates the 8
row blocks.
"""

import os
import sys

if "/opt/trn_rl_repo" not in sys.path:
    sys.path.insert(0, "/opt/trn_rl_repo")

import numpy as np
import ml_dtypes

import concourse.bass as bass
import concourse.tile as tile
from concourse import bacc, mybir
from concourse.bass_utils import run_bass_kernel_spmd
from concourse.masks import make_identity
from concourse.tile import add_dep_helper

BF16 = ml_dtypes.bfloat16

# Problem shape (hardcoded per contract).
T = 8192
HID = 2048
H = 16
D = 128
INNER = H * D
CHUNK = 128
NCHUNK = T // CHUNK  # 64
EPS = 1e-5
SCALE = D ** -0.5
ROPE_BASE = 600000.0
HALF = D // 2

N_CORES = 8
HPC = H // N_CORES          # 2 heads per core
CPC = HPC * D               # 256 inner cols per core
ROWS_PC = T // N_CORES      # 1024 output rows per core
RCHUNK = ROWS_PC // CHUNK   # 8 row-chunks per core in phase D

FP32 = mybir.dt.float32
BF = mybir.dt.bfloat16


def _build_program():
    nc = bacc.Bacc(
        "TRN2",
        target_bir_lowering=False,
        debug=False,
        num_devices=N_CORES,
    )

    # ---- DRAM I/O ----
    xt = nc.dram_tensor("xt", [NCHUNK, 128, HID // 128, CHUNK], BF,
                        kind="ExternalInput").ap()
    w = nc.dram_tensor("w", [128, HID // 128, 4 * CPC], BF,
                       kind="ExternalInput").ap()
    wo = nc.dram_tensor("wo", [128, HID // 128, HID], BF,
                        kind="ExternalInput").ap()
    qtab = nc.dram_tensor("qtab", [NCHUNK, CHUNK, 4 * D], BF,
                          kind="ExternalInput").ap()
    ktab = nc.dram_tensor("ktab", [NCHUNK, CHUNK, 4 * D], BF,
                          kind="ExternalInput").ap()
    maskt = nc.dram_tensor("maskt", [CHUNK, CHUNK], BF,
                           kind="ExternalInput").ap()
    out = nc.dram_tensor("out", [ROWS_PC, HID], FP32,
                         kind="ExternalOutput").ap()
    DEBUG = bool(os.environ.get("KB_DEBUG"))
    if DEBUG:
        dbg_o = nc.dram_tensor("dbg_o", [NCHUNK, 128, CPC], FP32,
                               kind="ExternalOutput").ap()
        dbg_ss = nc.dram_tensor("dbg_ss", [128, NCHUNK], FP32,
                                kind="ExternalOutput").ap()

    KO = HID // 128  # 16 k-chunks for the projections

    with tile.TileContext(nc) as tc:
        with (
            tc.tile_pool(name="const", bufs=1) as const,
            tc.tile_pool(name="persist", bufs=1) as persist,
            tc.tile_pool(name="dram", bufs=1, space="DRAM") as dram,
        ):
            # internal DRAM (A2A/RS split into 4 token-quarters so the
            # first three collectives overlap the chunk loop)
            TQ = T // 4
            a2a_in = [dram.tile([TQ // CHUNK, CPC, CHUNK], BF,
                                name=f"a2a_in{q}") for q in range(4)]
            a2a_out = [dram.tile([TQ // CHUNK, CPC, CHUNK], BF,
                                 name=f"a2a_out{q}") for q in range(4)]
            ss_part = [dram.tile([TQ], FP32, name=f"ss_part{q}")
                       for q in range(4)]
            ss_mine = [dram.tile([TQ // N_CORES], FP32, name=f"ss_mine{q}")
                       for q in range(4)]

            # prefetch chunk 0 inputs ahead of the weight loads
            pre_xt = const.tile([128, KO, CHUNK], BF)
            nc.sync.dma_start(out=pre_xt, in_=xt[0])
            pre_qtab = const.tile([CHUNK, 4 * D], BF)
            nc.sync.dma_start(out=pre_qtab, in_=qtab[0])
            pre_ktab = const.tile([CHUNK, 4 * D], BF)
            nc.sync.dma_start(out=pre_ktab, in_=ktab[0])

            # constants in SBUF
            w_s = const.tile([128, KO, 4 * CPC], BF)
            for ko in range(KO):
                nc.scalar.dma_start(out=w_s[:, ko, :], in_=w[:, ko, :])
            wo_s = const.tile([128, KO, HID], BF)
            maskt_s = const.tile([CHUNK, CHUNK], BF)
            nc.sync.dma_start(out=maskt_s, in_=maskt)
            ident = const.tile([128, 128], BF)
            make_identity(nc, ident)
            identf = const.tile([128, 128], FP32)
            make_identity(nc, identf)
            eps_t = const.tile([128, 1], FP32)
            nc.vector.memset(eps_t, EPS)

            # accumulators living across the whole chunk loop
            ssm = persist.tile([128, RCHUNK], FP32)
            rmsm = persist.tile([128, RCHUNK], FP32)
            rstd_m = persist.tile([128, RCHUNK], FP32)
            ss_all = persist.tile([128, NCHUNK], FP32)     # partial sumsq
            s_sb = persist.tile([128, HPC, D], BF)         # state for o_inter
            nc.vector.memset(s_sb, 0.0)
            aiv = a2a_in  # [q][lc, f, c] transposed payload

            # ---- phase A/B/C: projections + attention, per 128-token chunk ----
            with (
                tc.tile_pool(name="xt_p", bufs=3) as xt_p,
                tc.tile_pool(name="trig", bufs=2) as trig,
                tc.tile_pool(name="work", bufs=2) as work,
                tc.tile_pool(name="small", bufs=4) as small,
                tc.tile_pool(name="pq", bufs=2, space="PSUM") as pq,
                tc.tile_pool(name="ptr", bufs=2, space="PSUM") as ptr,
                tc.tile_pool(name="po", bufs=1, space="PSUM") as po,
                tc.tile_pool(name="ps", bufs=1, space="PSUM") as ps,
            ):
                # both heads' running states share one bank as a single
                # accumulation group: only the very first update (h=0, n=0)
                # carries start=True (clearing the bank-wide has_written
                # bits); h=1's first update overwrites its still-unwritten
                # region, and explicit deps keep that order on the PE
                s_psum_t = ps.tile([128, HPC * D], FP32)
                s_psum = [s_psum_t[:, h * D:(h + 1) * D] for h in range(HPC)]
                s_first_mm = [None]

                for n in range(NCHUNK):
                    # --- projection: qkvg chunk = x_chunk @ [Wq|Wk|Wv|Wg] ---
                    if n == 0:
                        xt_s = pre_xt
                    else:
                        xt_s = xt_p.tile([128, KO, CHUNK], BF)
                        xt_dma = nc.sync.dma_start(out=xt_s, in_=xt[n])
                        if n == 40:
                            xt40_dma = xt_dma
                        if n == NCHUNK - 1:
                            xt_last_dma = xt_dma
                    psum_qkvg = pq.tile([128, 4 * CPC], FP32)
                    for ko in range(KO):
                        for sl in range(2):
                            nc.tensor.matmul(
                                psum_qkvg[:, sl * 512:(sl + 1) * 512],
                                lhsT=xt_s[:, ko, :],
                                rhs=w_s[:, ko, sl * 512:(sl + 1) * 512],
                                start=(ko == 0),
                                stop=(ko == KO - 1),
                            )

                    # --- sigmoid of everything; silu(qkv) = qkv * sig ---
                    sig_f = work.tile([128, 4 * CPC], FP32, tag="sigf")
                    nc.scalar.activation(
                        out=sig_f, in_=psum_qkvg,
                        func=mybir.ActivationFunctionType.Sigmoid,
                    )
                    qkv_s = work.tile([128, 3 * CPC], FP32, tag="qkv")
                    nc.vector.tensor_mul(qkv_s, psum_qkvg[:, :3 * CPC],
                                         sig_f[:, :3 * CPC])

                    # --- per-head RMSNorm of q and k (over D=128) ---
                    sq_s = work.tile([128, 2 * CPC], FP32, tag="sq")
                    nc.vector.tensor_mul(sq_s, qkv_s[:, :2 * CPC],
                                         qkv_s[:, :2 * CPC])
                    ssum = small.tile([128, 4], FP32, tag="ssum")
                    nc.vector.reduce_sum(
                        out=ssum,
                        in_=sq_s.rearrange("p (g d) -> p g d", d=D),
                        axis=mybir.AxisListType.X,
                    )
                    rms = small.tile([128, 4], FP32, tag="rms")
                    nc.scalar.activation(
                        out=rms, in_=ssum,
                        func=mybir.ActivationFunctionType.Sqrt,
                        bias=eps_t, scale=1.0 / D,
                    )
                    rstd = small.tile([128, 4], FP32, tag="rstd")
                    nc.vector.reciprocal(out=rstd, in_=rms)
                    for g in range(4):
                        nc.vector.tensor_scalar_mul(
                            out=qkv_s[:, g * D:(g + 1) * D],
                            in0=qkv_s[:, g * D:(g + 1) * D],
                            scalar1=rstd[:, g:g + 1],
                        )

                    # --- rope on q and k (ln weights folded into tables:
                    # tab = [ln1*cos | ln2*sin | ln2*cos | ln1*sin]) ---
                    if n == 0:
                        qtab_t, ktab_t = pre_qtab, pre_ktab
                    else:
                        qtab_t = trig.tile([CHUNK, 4 * D], BF, tag="qtab")
                        nc.sync.dma_start(out=qtab_t, in_=qtab[n])
                        ktab_t = trig.tile([CHUNK, 4 * D], BF, tag="ktab")
                        nc.sync.dma_start(out=ktab_t, in_=ktab[n])

                    q_ro = work.tile([128, HPC, D], BF, tag="q_ro")
                    k_ro = work.tile([128, HPC, D], BF, tag="k_ro")
                    v_s = work.tile([128, HPC, D], BF, tag="v_s")
                    nc.vector.tensor_copy(v_s.rearrange("p h d -> p (h d)"),
                                          qkv_s[:, 2 * CPC:])
                    for src_off, tab, dst in ((0, qtab_t, q_ro),
                                              (CPC, ktab_t, k_ro)):
                        xv = qkv_s[:, src_off:src_off + CPC].rearrange(
                            "p (h d) -> p h d", d=D)
                        x1 = xv[:, :, :HALF]
                        x2 = xv[:, :, HALF:]
                        tv = tab.rearrange("p (i h f) -> p i h f", i=4, f=HALF)
                        t1 = work.tile([128, HPC, HALF], FP32, tag="ropet1")
                        t2 = work.tile([128, HPC, HALF], FP32, tag="ropet2")
                        nc.vector.tensor_mul(t1, x1, tv[:, 0])
                        nc.vector.tensor_mul(t2, x2, tv[:, 1])
                        nc.vector.tensor_sub(dst[:, :, :HALF], t1, t2)
                        nc.vector.tensor_mul(t1, x2, tv[:, 2])
                        nc.vector.tensor_mul(t2, x1, tv[:, 3])
                        nc.vector.tensor_add(dst[:, :, HALF:], t1, t2)

                    # --- transposes: qT, kT (per head) via PE ---
                    qT = work.tile([128, HPC, D], BF, tag="qT")
                    kT = work.tile([128, HPC, D], BF, tag="kT")
                    for ro, tr in ((q_ro, qT), (k_ro, kT)):
                        for h in range(HPC):
                            tp = ptr.tile([128, 128], BF, tag="tr")
                            nc.tensor.transpose(tp, ro[:, h, :], ident)
                            nc.vector.tensor_copy(tr[:, h, :], tp)

                    # --- attention ---
                    psum_o = po.tile([128, CPC], FP32)
                    prev_last_mm = None
                    for h in range(HPC):
                        sc_p = ptr.tile([128, 128], FP32, tag="tr")
                        nc.tensor.matmul(sc_p, lhsT=kT[:, h, :],
                                         rhs=qT[:, h, :], start=True, stop=True)
                        sc_s = work.tile([CHUNK, CHUNK], BF, tag="sc")
                        nc.vector.tensor_mul(sc_s, sc_p, maskt_s)
                        mm_intra = nc.tensor.matmul(
                            psum_o[:, h * D:(h + 1) * D],
                            lhsT=sc_s, rhs=v_s[:, h, :],
                            start=True, stop=(n == 0))
                        if prev_last_mm is not None:
                            # psum_o's bank is shared by both heads' groups;
                            # this start=True clears the whole bank's
                            # has_written bits, so it must not land between
                            # the other head's accumulating matmuls
                            add_dep_helper(mm_intra.ins, prev_last_mm.ins,
                                           reason="shared o-bank group order")
                        last_mm = mm_intra
                        if n > 0:
                            last_mm = nc.tensor.matmul(
                                psum_o[:, h * D:(h + 1) * D],
                                lhsT=qT[:, h, :], rhs=s_sb[:, h, :],
                                start=False, stop=True)
                        prev_last_mm = last_mm
                        if n < NCHUNK - 1:
                            mm_s = nc.tensor.matmul(
                                s_psum[h],
                                lhsT=k_ro[:, h, :],
                                rhs=v_s[:, h, :],
                                start=(n == 0 and h == 0),
                                stop=(n == NCHUNK - 2 and h == HPC - 1))
                            if n == 0:
                                if h == 0:
                                    s_first_mm[0] = mm_s
                                else:
                                    add_dep_helper(
                                        mm_s.ins, s_first_mm[0].ins,
                                        reason="bank clear before h1 write")
                            nc.vector.tensor_copy(s_sb[:, h, :], s_psum[h])

                    # --- g2 = o * (gnw * sig) -> a2a_in (rstd applied later,
                    # it commutes with the out-projection row-wise) ---
                    g2_t = work.tile([128, CPC], BF, tag="g2")
                    nc.vector.tensor_mul(g2_t, psum_o, sig_f[:, 3 * CPC:])
                    # ship the A2A payload pre-transposed so phase D needs
                    # only plain (non-XBAR) DMA loads
                    for h in range(HPC):
                        tp2 = ptr.tile([128, 128], BF, tag="tr",
                                       name=f"g2tp{n}_{h}")
                        nc.tensor.transpose(tp2, g2_t[:, h * D:(h + 1) * D],
                                            ident)
                        g2T = work.tile([128, CHUNK], BF, tag="g2T")
                        nc.vector.tensor_copy(g2T, tp2)
                        nc.scalar.dma_start(
                            out=aiv[n // 16][:][n % 16,
                                               h * D:(h + 1) * D, :],
                            in_=g2T)
                    # partial sumsq of o (f32, straight from psum)
                    osq = work.tile([128, CPC], FP32, tag="osq")
                    nc.scalar.activation(
                        out=osq, in_=psum_o,
                        func=mybir.ActivationFunctionType.Square,
                        accum_out=ss_all[:, n:n + 1],
                    )
                    if n >= 48:
                        # last quarter: store token-major immediately so RS3
                        # doesn't wait for a post-loop transpose
                        nc.scalar.dma_start(
                            out=ss_part[3][(n - 48) * CHUNK:
                                           (n - 47) * CHUNK],
                            in_=ss_all[:, n:n + 1])
                    if DEBUG:
                        do_ = work.tile([128, CPC], FP32, tag="dbgo")
                        nc.vector.tensor_copy(do_, psum_o)
                        nc.scalar.dma_start(out=dbg_o[n], in_=do_)

                    if 40 <= n < 40 + KO:
                        nc.sync.dma_start(out=wo_s[:, n - 40, :],
                                          in_=wo[:, n - 40, :])
                    if n % 16 == 15 and n < 48:
                        # quarter q complete: token-major ss slice, then
                        # A2A of the quarter's gated activations + RS of
                        # its sumsq -- the first three quarters' collectives
                        # overlap the remaining chunk loop
                        q = n // 16
                        ssT_p = ptr.tile([16, 128], FP32, tag="tr",
                                         name=f"ssT_p{q}")
                        nc.tensor.transpose(
                            ssT_p, ss_all[:, q * 16:(q + 1) * 16], identf)
                        ssT = small.tile([16, 128], FP32, tag="ssT",
                                         name=f"ssT{q}")
                        nc.vector.tensor_copy(ssT, ssT_p)
                        nc.scalar.dma_start(
                            out=ss_part[q][:].rearrange(
                                "(n c) -> n c", c=CHUNK),
                            in_=ssT)
                        rs_inst = nc.gpsimd.collective_compute(
                            "ReduceScatter",
                            mybir.AluOpType.add,
                            replica_groups=[list(range(N_CORES))],
                            ins=[ss_part[q][:].opt()],
                            outs=[ss_mine[q][:].opt()],
                        )
                        a2a_inst = nc.gpsimd.collective_compute(
                            "AllToAll",
                            mybir.AluOpType.bypass,
                            replica_groups=[list(range(N_CORES))],
                            ins=[a2a_in[q][:].opt()],
                            outs=[a2a_out[q][:].opt()],
                        )
                        add_dep_helper(a2a_inst.ins, rs_inst.ins, sync=False,
                                       reason="RS (tiny) ahead of A2A")
                        nc.gpsimd.dma_start(
                            out=ssm[:, q * 2:(q + 1) * 2],
                            in_=ss_mine[q][:].rearrange(
                                "(r c) -> c r", c=CHUNK))
                    if n == NCHUNK - 1:
                        q = 3
                        rs_inst = nc.gpsimd.collective_compute(
                            "ReduceScatter",
                            mybir.AluOpType.add,
                            replica_groups=[list(range(N_CORES))],
                            ins=[ss_part[q][:].opt()],
                            outs=[ss_mine[q][:].opt()],
                        )
                        a2a_inst = nc.gpsimd.collective_compute(
                            "AllToAll",
                            mybir.AluOpType.bypass,
                            replica_groups=[list(range(N_CORES))],
                            ins=[a2a_in[q][:].opt()],
                            outs=[a2a_out[q][:].opt()],
                        )
                        add_dep_helper(a2a_inst.ins, rs_inst.ins, sync=False,
                                       reason="RS (tiny) ahead of A2A")
                        nc.gpsimd.dma_start(
                            out=ssm[:, q * 2:(q + 1) * 2],
                            in_=ss_mine[q][:].rearrange(
                                "(r c) -> c r", c=CHUNK))
            if DEBUG:
                nc.scalar.dma_start(out=dbg_ss, in_=ss_all)


            def emit_rstd(q):
                nc.scalar.activation(
                    out=rmsm[:, q * 2:(q + 1) * 2],
                    in_=ssm[:, q * 2:(q + 1) * 2],
                    func=mybir.ActivationFunctionType.Sqrt,
                    bias=eps_t, scale=1.0 / INNER,
                )
                nc.vector.reciprocal(
                    out=rstd_m[:, q * 2:(q + 1) * 2],
                    in_=rmsm[:, q * 2:(q + 1) * 2])

            for q in range(3):
                emit_rstd(q)


            # ---- phase D: local out-projection of my token rows ----
            # a2a_out[q][j, lh, f, c] = gated^T block: cols j*CPC+f of
            # token q*2048 + me*256 + lh*128 + c
            aov = [a2a_out[q][:].rearrange("(j l) f c -> j l f c", j=N_CORES)
                   for q in range(4)]
            with (
                tc.tile_pool(name="dwork", bufs=2) as dwork,
                tc.tile_pool(name="gl", bufs=16) as gl,
                tc.tile_pool(name="pout", bufs=2, space="PSUM") as pout,
            ):
                for r in range(RCHUNK):
                    q, lh = r // 2, r % 2
                    if r == 6:
                        emit_rstd(3)
                    out_psum = pout.tile([128, HID], FP32)
                    for ko in range(KO):
                        j, half = ko // 2, ko % 2
                        glt = gl.tile([128, CHUNK], BF, tag="glt")
                        gl_dma = nc.sync.dma_start(
                            out=glt,
                            in_=aov[q][j, lh, half * 128:(half + 1) * 128, :],
                        )
                        if r == 0 and ko == 0:
                            # keep the transposed loads (serialized against
                            # collectives) out of the chunk loop's sync stream
                            add_dep_helper(gl_dma.ins, xt_last_dma.ins,
                                           sync=False,
                                           reason="gl loads after xt stream")
                        for sl in range(HID // 512):
                            nc.tensor.matmul(
                                out_psum[:, sl * 512:(sl + 1) * 512],
                                lhsT=glt,
                                rhs=wo_s[:, ko, sl * 512:(sl + 1) * 512],
                                start=(ko == 0),
                                stop=(ko == KO - 1),
                            )
                    out_sb = dwork.tile([128, HID], FP32, tag="out_sb")
                    nc.vector.tensor_scalar_mul(
                        out=out_sb[:, :HID // 2],
                        in0=out_psum[:, :HID // 2],
                        scalar1=rstd_m[:, r:r + 1])
                    nc.scalar.activation(
                        out=out_sb[:, HID // 2:], in_=out_psum[:, HID // 2:],
                        func=mybir.ActivationFunctionType.Copy,
                        scale=rstd_m[:, r:r + 1])
                    nc.scalar.dma_start(
                        out=out[r * CHUNK:(r + 1) * CHUNK, :], in_=out_sb)

    nc.compile()
    return nc


_PROGRAM = None


def _get_program():
    global _PROGRAM
    if _PROGRAM is None:
        _PROGRAM = _build_program()
    return _PROGRAM


def _prep_inputs(hidden_states, positions, Wqkv, q_ln_w, k_ln_w, Wg, g_norm_w,
                 Wo):
    x = np.asarray(hidden_states, dtype=np.float32)
    pos = np.asarray(positions).astype(np.float32)

    # x transposed+tiled: xt[n, p, ko, t] = x[n*128+t, ko*128+p]
    xt = np.ascontiguousarray(
        x.reshape(NCHUNK, CHUNK, HID // 128, 128).transpose(0, 3, 2, 1)
    ).astype(BF16)

    # rope tables with the per-head ln weights (and q's 1/sqrt(D) scale)
    # folded in: tab = [ln1*cos | ln2*sin | ln2*cos | ln1*sin], head-dup'd
    inv_freq = (1.0 / (ROPE_BASE ** (np.arange(HALF, dtype=np.float32) / HALF))
                ).astype(np.float32)
    ang = pos[:, None] * inv_freq[None, :]              # [T, HALF]
    cos = np.cos(ang).astype(np.float32)
    sin = np.sin(ang).astype(np.float32)

    def rope_tab(ln, scale):
        l1 = (np.asarray(ln, np.float32)[:HALF] * scale)[None, :]
        l2 = (np.asarray(ln, np.float32)[HALF:] * scale)[None, :]
        tab = np.stack([l1 * cos, l2 * sin, l2 * cos, l1 * sin], axis=1)
        tab = np.tile(tab[:, :, None, :], (1, 1, 2, 1))   # dup per head
        return tab.reshape(NCHUNK, CHUNK, 4 * D).astype(BF16)

    qtab = rope_tab(q_ln_w, SCALE)
    ktab = rope_tab(k_ln_w, 1.0)

    maskt = np.triu(np.ones((CHUNK, CHUNK), dtype=np.float32)).astype(BF16)

    Wqkv = np.asarray(Wqkv, np.float32)
    Wg_ = np.asarray(Wg, np.float32)
    Wo_ = np.asarray(Wo, np.float32)
    gn = np.asarray(g_norm_w, np.float32)

    # full Wo with g_norm_w folded into its rows, [p, ko, n] layout
    Wo_g = Wo_ * gn[:, None]
    wo_r = np.ascontiguousarray(
        Wo_g.reshape(HID // 128, 128, HID).transpose(1, 0, 2)
    ).astype(BF16)

    in_maps = []
    for c in range(N_CORES):
        lo, hi = c * CPC, (c + 1) * CPC
        wc = np.concatenate(
            [Wqkv[:, lo:hi], Wqkv[:, INNER + lo:INNER + hi],
             Wqkv[:, 2 * INNER + lo:2 * INNER + hi], Wg_[:, lo:hi]], axis=1)
        w_r = np.ascontiguousarray(
            wc.reshape(HID // 128, 128, 4 * CPC).transpose(1, 0, 2)
        ).astype(BF16)
        in_maps.append({
            "xt": xt, "w": w_r, "wo": wo_r, "qtab": qtab, "ktab": ktab,
            "maskt": maskt,
        })
    return in_maps


def run(trace=False, **inputs):
    nc = _get_program()
    in_maps = _prep_inputs(**inputs)
    res = run_bass_kernel_spmd(nc, in_maps, list(range(N_CORES)), trace=trace)
    # core c's row i = (2q+lh)*128 + c128  <->  token q*2048 + c*256
    #                                               + lh*128 + c128
    stacked = np.stack([res.results[c]["out"] for c in range(N_CORES)])
    full = (stacked.reshape(N_CORES, 4, 2, CHUNK, HID)
            .transpose(1, 0, 2, 3, 4).reshape(T, HID))
    return full, res


def kernel(**inputs):
    full, _ = run(trace=False, **inputs)
    return full


# revision 23
# speedup vs baseline: 1.0374x; 1.0374x over previous
"""Bass/Trainium2 kernel for nn_BailingMoELinearAttention.

Tensor-parallel over heads across 8 NeuronCores: each core owns 2 of the 16
heads (columns of Wqkv/Wg, rows of Wo). Per-core pipeline, fused per 128-token
chunk:
  qkvg projection (bf16 matmul) -> silu(q,k,v) / sigmoid(gate) -> per-head
  RMSNorm -> RoPE (host-precomputed cos/sin tables) -> chunked causal linear
  attention (running k^T v state, one PSUM bank per head) -> pre-gated
  activations g2 = o * g_norm_w * sigmoid(gate) and partial sum-of-squares.
Cross-core: AllReduce of per-token sum-of-squares (the group norm spans all 16
heads), per-token rstd scaling, then an AllToAll that exchanges the bf16 gated
activations so each core holds all 2048 inner columns for its 1024-token row
block, and a local out-projection against the full Wo. Host concatenates the 8
row blocks.
"""

import os
import sys

if "/opt/trn_rl_repo" not in sys.path:
    sys.path.insert(0, "/opt/trn_rl_repo")

import numpy as np
import ml_dtypes

import concourse.bass as bass
import concourse.tile as tile
from concourse import bacc, mybir
from concourse.bass_utils import run_bass_kernel_spmd
from concourse.masks import make_identity
from concourse.tile import add_dep_helper

BF16 = ml_dtypes.bfloat16

# Problem shape (hardcoded per contract).
T = 8192
HID = 2048
H = 16
D = 128
INNER = H * D
CHUNK = 128
NCHUNK = T // CHUNK  # 64
EPS = 1e-5
SCALE = D ** -0.5
ROPE_BASE = 600000.0
HALF = D // 2

N_CORES = 8
HPC = H // N_CORES          # 2 heads per core
CPC = HPC * D               # 256 inner cols per core
ROWS_PC = T // N_CORES      # 1024 output rows per core
RCHUNK = ROWS_PC // CHUNK   # 8 row-chunks per core in phase D

FP32 = mybir.dt.float32
BF = mybir.dt.bfloat16


def _build_program():
    nc = bacc.Bacc(
        "TRN2",
        target_bir_lowering=False,
        debug=False,
        num_devices=N_CORES,
    )

    # ---- DRAM I/O ----
    xt = nc.dram_tensor("xt", [NCHUNK, 128, HID // 128, CHUNK], BF,
                        kind="ExternalInput").ap()
    w = nc.dram_tensor("w", [128, HID // 128, 4 * CPC], BF,
                       kind="ExternalInput").ap()
    wo = nc.dram_tensor("wo", [128, HID // 128, HID], BF,
                        kind="ExternalInput").ap()
    qtab = nc.dram_tensor("qtab", [NCHUNK, CHUNK, 4 * D], BF,
                          kind="ExternalInput").ap()
    ktab = nc.dram_tensor("ktab", [NCHUNK, CHUNK, 4 * D], BF,
                          kind="ExternalInput").ap()
    maskt = nc.dram_tensor("maskt", [CHUNK, CHUNK], BF,
                           kind="ExternalInput").ap()
    out = nc.dram_tensor("out", [ROWS_PC, HID], FP32,
                         kind="ExternalOutput").ap()
    DEBUG = bool(os.environ.get("KB_DEBUG"))
    if DEBUG:
        dbg_o = nc.dram_tensor("dbg_o", [NCHUNK, 128, CPC], FP32,
                               kind="ExternalOutput").ap()
        dbg_ss = nc.dram_tensor("dbg_ss", [128, NCHUNK], FP32,
                                kind="ExternalOutput").ap()

    KO = HID // 128  # 16 k-chunks for the projections

    with tile.TileContext(nc) as tc:
        with (
            tc.tile_pool(name="const", bufs=1) as const,
            tc.tile_pool(name="persist", bufs=1) as persist,
            tc.tile_pool(name="dram", bufs=1, space="DRAM") as dram,
        ):
            # internal DRAM (A2A/RS split into 4 token-quarters so the
            # first three collectives overlap the chunk loop)
            TQ = T // 4
            a2a_in = [dram.tile([TQ // CHUNK, CPC, CHUNK], BF,
                                name=f"a2a_in{q}") for q in range(4)]
            a2a_out = [dram.tile([TQ // CHUNK, CPC, CHUNK], BF,
                                 name=f"a2a_out{q}") for q in range(4)]
            ss_part = [dram.tile([TQ], FP32, name=f"ss_part{q}")
                       for q in range(4)]
            ss_mine = [dram.tile([TQ // N_CORES], FP32, name=f"ss_mine{q}")
                       for q in range(4)]

            # prefetch chunk 0 inputs ahead of the weight loads
            pre_xt = const.tile([128, KO, CHUNK], BF)
            nc.sync.dma_start(out=pre_xt, in_=xt[0])
            pre_qtab = const.tile([CHUNK, 4 * D], BF)
            nc.sync.dma_start(out=pre_qtab, in_=qtab[0])
            pre_ktab = const.tile([CHUNK, 4 * D], BF)
            nc.sync.dma_start(out=pre_ktab, in_=ktab[0])

            # constants in SBUF
            w_s = const.tile([128, KO, 4 * CPC], BF)
            for ko in range(KO):
                nc.scalar.dma_start(out=w_s[:, ko, :], in_=w[:, ko, :])
            wo_s = const.tile([128, KO, HID], BF)
            maskt_s = const.tile([CHUNK, CHUNK], BF)
            nc.sync.dma_start(out=maskt_s, in_=maskt)
            ident = const.tile([128, 128], BF)
            make_identity(nc, ident)
            identf = const.tile([128, 128], FP32)
            make_identity(nc, identf)
            eps_t = const.tile([128, 1], FP32)
            nc.vector.memset(eps_t, EPS)

            # accumulators living across the whole chunk loop
            ssm = persist.tile([128, RCHUNK], FP32)
            rmsm = persist.tile([128, RCHUNK], FP32)
            rstd_m = persist.tile([128, RCHUNK], FP32)
            ss_all = persist.tile([128, NCHUNK], FP32)     # partial sumsq
            s_sb = persist.tile([128, HPC, D], BF)         # state for o_inter
            nc.vector.memset(s_sb, 0.0)
            aiv = a2a_in  # [q][lc, f, c] transposed payload

            # ---- phase A/B/C: projections + attention, per 128-token chunk ----
            with (
                tc.tile_pool(name="xt_p", bufs=3) as xt_p,
                tc.tile_pool(name="trig", bufs=2) as trig,
                tc.tile_pool(name="work", bufs=2) as work,
                tc.tile_pool(name="small", bufs=4) as small,
                tc.tile_pool(name="pq", bufs=2, space="PSUM") as pq,
                tc.tile_pool(name="ptr", bufs=1, space="PSUM") as ptr,
                tc.tile_pool(name="po", bufs=1, space="PSUM") as po,
                tc.tile_pool(name="ps", bufs=1, space="PSUM") as ps,
            ):
                # one PSUM bank per head: a start=True from another group
                # sharing the bank would clear this group's has_written bits
                # and break the running accumulation
                s_psum = [ps.tile([128, D], FP32, tag=f"s{h}",
                                  name=f"s_psum{h}")
                          for h in range(HPC)]

                for n in range(NCHUNK):
                    # --- projection: qkvg chunk = x_chunk @ [Wq|Wk|Wv|Wg] ---
                    if n == 0:
                        xt_s = pre_xt
                    else:
                        xt_s = xt_p.tile([128, KO, CHUNK], BF)
                        xt_dma = nc.sync.dma_start(out=xt_s, in_=xt[n])
                        if n == 40:
                            xt40_dma = xt_dma
                        if n == NCHUNK - 1:
                            xt_last_dma = xt_dma
                    psum_qkvg = pq.tile([128, 4 * CPC], FP32)
                    for ko in range(KO):
                        for sl in range(2):
                            nc.tensor.matmul(
                                psum_qkvg[:, sl * 512:(sl + 1) * 512],
                                lhsT=xt_s[:, ko, :],
                                rhs=w_s[:, ko, sl * 512:(sl + 1) * 512],
                                start=(ko == 0),
                                stop=(ko == KO - 1),
                            )

                    # --- sigmoid of everything; silu(qkv) = qkv * sig ---
                    sig_f = work.tile([128, 4 * CPC], FP32, tag="sigf")
                    nc.scalar.activation(
                        out=sig_f, in_=psum_qkvg,
                        func=mybir.ActivationFunctionType.Sigmoid,
                    )
                    qkv_s = work.tile([128, 3 * CPC], FP32, tag="qkv")
                    nc.vector.tensor_mul(qkv_s, psum_qkvg[:, :3 * CPC],
                                         sig_f[:, :3 * CPC])

                    # --- per-head RMSNorm of q and k (over D=128) ---
                    sq_s = work.tile([128, 2 * CPC], FP32, tag="sq")
                    nc.vector.tensor_mul(sq_s, qkv_s[:, :2 * CPC],
                                         qkv_s[:, :2 * CPC])
                    ssum = small.tile([128, 4], FP32, tag="ssum")
                    nc.vector.reduce_sum(
                        out=ssum,
                        in_=sq_s.rearrange("p (g d) -> p g d", d=D),
                        axis=mybir.AxisListType.X,
                    )
                    rms = small.tile([128, 4], FP32, tag="rms")
                    nc.scalar.activation(
                        out=rms, in_=ssum,
                        func=mybir.ActivationFunctionType.Sqrt,
                        bias=eps_t, scale=1.0 / D,
                    )
                    rstd = small.tile([128, 4], FP32, tag="rstd")
                    nc.vector.reciprocal(out=rstd, in_=rms)
                    for g in range(4):
                        nc.vector.tensor_scalar_mul(
                            out=qkv_s[:, g * D:(g + 1) * D],
                            in0=qkv_s[:, g * D:(g + 1) * D],
                            scalar1=rstd[:, g:g + 1],
                        )

                    # --- rope on q and k (ln weights folded into tables:
                    # tab = [ln1*cos | ln2*sin | ln2*cos | ln1*sin]) ---
                    if n == 0:
                        qtab_t, ktab_t = pre_qtab, pre_ktab
                    else:
                        qtab_t = trig.tile([CHUNK, 4 * D], BF, tag="qtab")
                        nc.sync.dma_start(out=qtab_t, in_=qtab[n])
                        ktab_t = trig.tile([CHUNK, 4 * D], BF, tag="ktab")
                        nc.sync.dma_start(out=ktab_t, in_=ktab[n])

                    q_ro = work.tile([128, HPC, D], BF, tag="q_ro")
                    k_ro = work.tile([128, HPC, D], BF, tag="k_ro")
                    v_s = work.tile([128, HPC, D], BF, tag="v_s")
                    nc.vector.tensor_copy(v_s.rearrange("p h d -> p (h d)"),
                                          qkv_s[:, 2 * CPC:])
                    for src_off, tab, dst in ((0, qtab_t, q_ro),
                                              (CPC, ktab_t, k_ro)):
                        xv = qkv_s[:, src_off:src_off + CPC].rearrange(
                            "p (h d) -> p h d", d=D)
                        x1 = xv[:, :, :HALF]
                        x2 = xv[:, :, HALF:]
                        tv = tab.rearrange("p (i h f) -> p i h f", i=4, f=HALF)
                        t1 = work.tile([128, HPC, HALF], FP32, tag="ropet1")
                        t2 = work.tile([128, HPC, HALF], FP32, tag="ropet2")
                        nc.vector.tensor_mul(t1, x1, tv[:, 0])
                        nc.vector.tensor_mul(t2, x2, tv[:, 1])
                        nc.vector.tensor_sub(dst[:, :, :HALF], t1, t2)
                        nc.vector.tensor_mul(t1, x2, tv[:, 2])
                        nc.vector.tensor_mul(t2, x1, tv[:, 3])
                        nc.vector.tensor_add(dst[:, :, HALF:], t1, t2)

                    # --- transposes: qT, kT (per head) via PE ---
                    qT = work.tile([128, HPC, D], BF, tag="qT")
                    kT = work.tile([128, HPC, D], BF, tag="kT")
                    for ro, tr in ((q_ro, qT), (k_ro, kT)):
                        for h in range(HPC):
                            tp = ptr.tile([128, 128], BF, tag="tr")
                            nc.tensor.transpose(tp, ro[:, h, :], ident)
                            nc.vector.tensor_copy(tr[:, h, :], tp)

                    # --- attention ---
                    psum_o = po.tile([128, CPC], FP32)
                    prev_last_mm = None
                    for h in range(HPC):
                        sc_p = ptr.tile([128, 128], FP32, tag="tr")
                        nc.tensor.matmul(sc_p, lhsT=kT[:, h, :],
                                         rhs=qT[:, h, :], start=True, stop=True)
                        sc_s = work.tile([CHUNK, CHUNK], BF, tag="sc")
                        nc.vector.tensor_mul(sc_s, sc_p, maskt_s)
                        mm_intra = nc.tensor.matmul(
                            psum_o[:, h * D:(h + 1) * D],
                            lhsT=sc_s, rhs=v_s[:, h, :],
                            start=True, stop=(n == 0))
                        if prev_last_mm is not None:
                            # psum_o's bank is shared by both heads' groups;
                            # this start=True clears the whole bank's
                            # has_written bits, so it must not land between
                            # the other head's accumulating matmuls
                            add_dep_helper(mm_intra.ins, prev_last_mm.ins,
                                           reason="shared o-bank group order")
                        last_mm = mm_intra
                        if n > 0:
                            last_mm = nc.tensor.matmul(
                                psum_o[:, h * D:(h + 1) * D],
                                lhsT=qT[:, h, :], rhs=s_sb[:, h, :],
                                start=False, stop=True)
                        prev_last_mm = last_mm
                        if n < NCHUNK - 1:
                            nc.tensor.matmul(s_psum[h],
                                             lhsT=k_ro[:, h, :],
                                             rhs=v_s[:, h, :],
                                             start=(n == 0),
                                             stop=(n == NCHUNK - 2))
                            nc.vector.tensor_copy(s_sb[:, h, :], s_psum[h])

                    # --- g2 = o * (gnw * sig) -> a2a_in (rstd applied later,
                    # it commutes with the out-projection row-wise) ---
                    g2_t = work.tile([128, CPC], BF, tag="g2")
                    nc.vector.tensor_mul(g2_t, psum_o, sig_f[:, 3 * CPC:])
                    # ship the A2A payload pre-transposed so phase D needs
                    # only plain (non-XBAR) DMA loads
                    for h in range(HPC):
                        tp2 = ptr.tile([128, 128], BF, tag="tr",
                                       name=f"g2tp{n}_{h}")
                        nc.tensor.transpose(tp2, g2_t[:, h * D:(h + 1) * D],
                                            ident)
                        g2T = work.tile([128, CHUNK], BF, tag="g2T")
                        nc.vector.tensor_copy(g2T, tp2)
                        nc.scalar.dma_start(
                            out=aiv[n // 16][:][n % 16,
                                               h * D:(h + 1) * D, :],
                            in_=g2T)
                    # partial sumsq of o (f32, straight from psum)
                    osq = work.tile([128, CPC], FP32, tag="osq")
                    nc.scalar.activation(
                        out=osq, in_=psum_o,
                        func=mybir.ActivationFunctionType.Square,
                        accum_out=ss_all[:, n:n + 1],
                    )
                    if n >= 48:
                        # last quarter: store token-major immediately so RS3
                        # doesn't wait for a post-loop transpose
                        nc.scalar.dma_start(
                            out=ss_part[3][(n - 48) * CHUNK:
                                           (n - 47) * CHUNK],
                            in_=ss_all[:, n:n + 1])
                    if DEBUG:
                        do_ = work.tile([128, CPC], FP32, tag="dbgo")
                        nc.vector.tensor_copy(do_, psum_o)
                        nc.scalar.dma_start(out=dbg_o[n], in_=do_)

                    if 40 <= n < 40 + KO:
                        nc.sync.dma_start(out=wo_s[:, n - 40, :],
                                          in_=wo[:, n - 40, :])
                    if n % 16 == 15 and n < 48:
                        # quarter q complete: token-major ss slice, then
                        # A2A of the quarter's gated activations + RS of
                        # its sumsq -- the first three quarters' collectives
                        # overlap the remaining chunk loop
                        q = n // 16
                        ssT_p = ptr.tile([16, 128], FP32, tag="tr",
                                         name=f"ssT_p{q}")
                        nc.tensor.transpose(
                            ssT_p, ss_all[:, q * 16:(q + 1) * 16], identf)
                        ssT = small.tile([16, 128], FP32, tag="ssT",
                                         name=f"ssT{q}")
                        nc.vector.tensor_copy(ssT, ssT_p)
                        nc.scalar.dma_start(
                            out=ss_part[q][:].rearrange(
                                "(n c) -> n c", c=CHUNK),
                            in_=ssT)
                        rs_inst = nc.gpsimd.collective_compute(
                            "ReduceScatter",
                            mybir.AluOpType.add,
                            replica_groups=[list(range(N_CORES))],
                            ins=[ss_part[q][:].opt()],
                            outs=[ss_mine[q][:].opt()],
                        )
                        a2a_inst = nc.gpsimd.collective_compute(
                            "AllToAll",
                            mybir.AluOpType.bypass,
                            replica_groups=[list(range(N_CORES))],
                            ins=[a2a_in[q][:].opt()],
                            outs=[a2a_out[q][:].opt()],
                        )
                        add_dep_helper(a2a_inst.ins, rs_inst.ins, sync=False,
                                       reason="RS (tiny) ahead of A2A")
                        nc.gpsimd.dma_start(
                            out=ssm[:, q * 2:(q + 1) * 2],
                            in_=ss_mine[q][:].rearrange(
                                "(r c) -> c r", c=CHUNK))
                    if n == NCHUNK - 1:
                        q = 3
                        rs_inst = nc.gpsimd.collective_compute(
                            "ReduceScatter",
                            mybir.AluOpType.add,
                            replica_groups=[list(range(N_CORES))],
                            ins=[ss_part[q][:].opt()],
                            outs=[ss_mine[q][:].opt()],
                        )
                        a2a_inst = nc.gpsimd.collective_compute(
                            "AllToAll",
                            mybir.AluOpType.bypass,
                            replica_groups=[list(range(N_CORES))],
                            ins=[a2a_in[q][:].opt()],
                            outs=[a2a_out[q][:].opt()],
                        )
                        add_dep_helper(a2a_inst.ins, rs_inst.ins, sync=False,
                                       reason="RS (tiny) ahead of A2A")
                        nc.gpsimd.dma_start(
                            out=ssm[:, q * 2:(q + 1) * 2],
                            in_=ss_mine[q][:].rearrange(
                                "(r c) -> c r", c=CHUNK))
            if DEBUG:
                nc.scalar.dma_start(out=dbg_ss, in_=ss_all)


            def emit_rstd(q):
                nc.scalar.activation(
                    out=rmsm[:, q * 2:(q + 1) * 2],
                    in_=ssm[:, q * 2:(q + 1) * 2],
                    func=mybir.ActivationFunctionType.Sqrt,
                    bias=eps_t, scale=1.0 / INNER,
                )
                nc.vector.reciprocal(
                    out=rstd_m[:, q * 2:(q + 1) * 2],
                    in_=rmsm[:, q * 2:(q + 1) * 2])

            for q in range(3):
                emit_rstd(q)


            # ---- phase D: local out-projection of my token rows ----
            # a2a_out[q][j, lh, f, c] = gated^T block: cols j*CPC+f of
            # token q*2048 + me*256 + lh*128 + c
            aov = [a2a_out[q][:].rearrange("(j l) f c -> j l f c", j=N_CORES)
                   for q in range(4)]
            with (
                tc.tile_pool(name="dwork", bufs=2) as dwork,
                tc.tile_pool(name="gl", bufs=16) as gl,
                tc.tile_pool(name="pout", bufs=2, space="PSUM") as pout,
            ):
                for r in range(RCHUNK):
                    q, lh = r // 2, r % 2
                    if r == 6:
                        emit_rstd(3)
                    out_psum = pout.tile([128, HID], FP32)
                    for ko in range(KO):
                        j, half = ko // 2, ko % 2
                        glt = gl.tile([128, CHUNK], BF, tag="glt")
                        gl_dma = nc.sync.dma_start(
                            out=glt,
                            in_=aov[q][j, lh, half * 128:(half + 1) * 128, :],
                        )
                        if r == 0 and ko == 0:
                            # keep the transposed loads (serialized against
                            # collectives) out of the chunk loop's sync stream
                            add_dep_helper(gl_dma.ins, xt_last_dma.ins,
                                           sync=False,
                                           reason="gl loads after xt stream")
                        for sl in range(HID // 512):
                            nc.tensor.matmul(
                                out_psum[:, sl * 512:(sl + 1) * 512],
                                lhsT=glt,
                                rhs=wo_s[:, ko, sl * 512:(sl + 1) * 512],
                                start=(ko == 0),
                                stop=(ko == KO - 1),
                            )
                    out_sb = dwork.tile([128, HID], FP32, tag="out_sb")
                    nc.vector.tensor_scalar_mul(
                        out=out_sb[:, :HID // 2],
                        in0=out_psum[:, :HID // 2],
                        scalar1=rstd_m[:, r:r + 1])
                    nc.scalar.activation(
                        out=out_sb[:, HID // 2:], in_=out_psum[:, HID // 2:],
                        func=mybir.ActivationFunctionType.Copy,
                        scale=rstd_m[:, r:r + 1])
                    nc.scalar.dma_start(
                        out=out[r * CHUNK:(r + 1) * CHUNK, :], in_=out_sb)

    nc.compile()
    return nc


_PROGRAM = None


def _get_program():
    global _PROGRAM
    if _PROGRAM is None:
        _PROGRAM = _build_program()
    return _PROGRAM


def _prep_inputs(hidden_states, positions, Wqkv, q_ln_w, k_ln_w, Wg, g_norm_w,
                 Wo):
    x = np.asarray(hidden_states, dtype=np.float32)
    pos = np.asarray(positions).astype(np.float32)

    # x transposed+tiled: xt[n, p, ko, t] = x[n*128+t, ko*128+p]
    xt = np.ascontiguousarray(
        x.reshape(NCHUNK, CHUNK, HID // 128, 128).transpose(0, 3, 2, 1)
    ).astype(BF16)

    # rope tables with the per-head ln weights (and q's 1/sqrt(D) scale)
    # folded in: tab = [ln1*cos | ln2*sin | ln2*cos | ln1*sin], head-dup'd
    inv_freq = (1.0 / (ROPE_BASE ** (np.arange(HALF, dtype=np.float32) / HALF))
                ).astype(np.float32)
    ang = pos[:, None] * inv_freq[None, :]              # [T, HALF]
    cos = np.cos(ang).astype(np.float32)
    sin = np.sin(ang).astype(np.float32)

    def rope_tab(ln, scale):
        l1 = (np.asarray(ln, np.float32)[:HALF] * scale)[None, :]
        l2 = (np.asarray(ln, np.float32)[HALF:] * scale)[None, :]
        tab = np.stack([l1 * cos, l2 * sin, l2 * cos, l1 * sin], axis=1)
        tab = np.tile(tab[:, :, None, :], (1, 1, 2, 1))   # dup per head
        return tab.reshape(NCHUNK, CHUNK, 4 * D).astype(BF16)

    qtab = rope_tab(q_ln_w, SCALE)
    ktab = rope_tab(k_ln_w, 1.0)

    maskt = np.triu(np.ones((CHUNK, CHUNK), dtype=np.float32)).astype(BF16)

    Wqkv = np.asarray(Wqkv, np.float32)
    Wg_ = np.asarray(Wg, np.float32)
    Wo_ = np.asarray(Wo, np.float32)
    gn = np.asarray(g_norm_w, np.float32)

    # full Wo with g_norm_w folded into its rows, [p, ko, n] layout
    Wo_g = Wo_ * gn[:, None]
    wo_r = np.ascontiguousarray(
        Wo_g.reshape(HID // 128, 128, HID).transpose(1, 0, 2)
    ).astype(BF16)

    in_maps = []
    for c in range(N_CORES):
        lo, hi = c * CPC, (c + 1) * CPC
        wc = np.concatenate(
            [Wqkv[:, lo:hi], Wqkv[:, INNER + lo:INNER + hi],
             Wqkv[:, 2 * INNER + lo:2 * INNER + hi], Wg_[:, lo:hi]], axis=1)
        w_r = np.ascontiguousarray(
            wc.reshape(HID // 128, 128, 4 * CPC).transpose(1, 0, 2)
        ).astype(BF16)
        in_maps.append({
            "xt": xt, "w": w_r, "wo": wo_r, "qtab": qtab, "ktab": ktab,
            "maskt": maskt,
        })
    return in_maps


def run(trace=False, **inputs):
    nc = _get_program()
    in_maps = _prep_inputs(**inputs)
    res = run_bass_kernel_spmd(nc, in_maps, list(range(N_CORES)), trace=trace)
    # core c's row i = (2q+lh)*128 + c128  <->  token q*2048 + c*256
    #                                               + lh*128 + c128
    stacked = np.stack([res.results[c]["out"] for c in range(N_CORES)])
    full = (stacked.reshape(N_CORES, 4, 2, CHUNK, HID)
            .transpose(1, 0, 2, 3, 4).reshape(T, HID))
    return full, res


def kernel(**inputs):
    full, _ = run(trace=False, **inputs)
    return full


# revision 24
# speedup vs baseline: 1.1464x; 1.1051x over previous
"""Bass/Trainium2 kernel for nn_BailingMoELinearAttention.

Tensor-parallel over heads across 8 NeuronCores: each core owns 2 of the 16
heads (columns of Wqkv/Wg, rows of Wo). Per-core pipeline, fused per 128-token
chunk:
  qkvg projection (bf16 matmul) -> silu(q,k,v) / sigmoid(gate) -> per-head
  RMSNorm -> RoPE (host-precomputed cos/sin tables) -> chunked causal linear
  attention (running k^T v state, one PSUM bank per head) -> pre-gated
  activations g2 = o * g_norm_w * sigmoid(gate) and partial sum-of-squares.
Cross-core: AllReduce of per-token sum-of-squares (the group norm spans all 16
heads), per-token rstd scaling, then an AllToAll that exchanges the bf16 gated
activations so each core holds all 2048 inner columns for its 1024-token row
block, and a local out-projection against the full Wo. Host concatenates the 8
row blocks.
"""

import os
import sys

if "/opt/trn_rl_repo" not in sys.path:
    sys.path.insert(0, "/opt/trn_rl_repo")

import numpy as np
import ml_dtypes

import concourse.bass as bass
import concourse.tile as tile
from concourse import bacc, mybir
from concourse.bass_utils import run_bass_kernel_spmd
from concourse.masks import make_identity
from concourse.tile import add_dep_helper

BF16 = ml_dtypes.bfloat16

# Problem shape (hardcoded per contract).
T = 8192
HID = 2048
H = 16
D = 128
INNER = H * D
CHUNK = 128
NCHUNK = T // CHUNK  # 64
EPS = 1e-5
SCALE = D ** -0.5
ROPE_BASE = 600000.0
HALF = D // 2

N_CORES = 8
HPC = H // N_CORES          # 2 heads per core
CPC = HPC * D               # 256 inner cols per core
ROWS_PC = T // N_CORES      # 1024 output rows per core
RCHUNK = ROWS_PC // CHUNK   # 8 row-chunks per core in phase D

FP32 = mybir.dt.float32
BF = mybir.dt.bfloat16


def _build_program():
    nc = bacc.Bacc(
        "TRN2",
        target_bir_lowering=False,
        debug=False,
        num_devices=N_CORES,
    )

    # ---- DRAM I/O ----
    xt = nc.dram_tensor("xt", [NCHUNK, 128, HID // 128, CHUNK], BF,
                        kind="ExternalInput").ap()
    w = nc.dram_tensor("w", [128, HID // 128, 4 * CPC], BF,
                       kind="ExternalInput").ap()
    wo = nc.dram_tensor("wo", [128, HID // 128, HID], BF,
                        kind="ExternalInput").ap()
    qtab = nc.dram_tensor("qtab", [NCHUNK, CHUNK, 4 * D], BF,
                          kind="ExternalInput").ap()
    ktab = nc.dram_tensor("ktab", [NCHUNK, CHUNK, 4 * D], BF,
                          kind="ExternalInput").ap()
    maskt = nc.dram_tensor("maskt", [CHUNK, CHUNK], BF,
                           kind="ExternalInput").ap()
    out = nc.dram_tensor("out", [ROWS_PC, HID], FP32,
                         kind="ExternalOutput").ap()
    DEBUG = bool(os.environ.get("KB_DEBUG"))
    if DEBUG:
        dbg_o = nc.dram_tensor("dbg_o", [NCHUNK, 128, CPC], FP32,
                               kind="ExternalOutput").ap()
        dbg_ss = nc.dram_tensor("dbg_ss", [128, NCHUNK], FP32,
                                kind="ExternalOutput").ap()

    KO = HID // 128  # 16 k-chunks for the projections

    with tile.TileContext(nc) as tc:
        with (
            tc.tile_pool(name="const", bufs=1) as const,
            tc.tile_pool(name="persist", bufs=1) as persist,
            tc.tile_pool(name="dram", bufs=1, space="DRAM") as dram,
        ):
            # internal DRAM (A2A/RS split into 4 token-quarters so the
            # first three collectives overlap the chunk loop)
            TQ = T // 4
            a2a_in = [dram.tile([TQ // CHUNK, CPC, CHUNK], BF,
                                name=f"a2a_in{q}") for q in range(4)]
            a2a_out = [dram.tile([TQ // CHUNK, CPC, CHUNK], BF,
                                 name=f"a2a_out{q}") for q in range(4)]
            ss_part = [dram.tile([TQ], FP32, name=f"ss_part{q}")
                       for q in range(4)]
            ss_mine = [dram.tile([TQ // N_CORES], FP32, name=f"ss_mine{q}")
                       for q in range(4)]

            # prefetch chunk 0 inputs ahead of the weight loads
            pre_xt = const.tile([128, KO, CHUNK], BF)
            nc.sync.dma_start(out=pre_xt, in_=xt[0])
            pre_qtab = const.tile([CHUNK, 4 * D], BF)
            nc.sync.dma_start(out=pre_qtab, in_=qtab[0])
            pre_ktab = const.tile([CHUNK, 4 * D], BF)
            nc.sync.dma_start(out=pre_ktab, in_=ktab[0])

            # constants in SBUF
            w_s = const.tile([128, KO, 4 * CPC], BF)
            for ko in range(KO):
                nc.scalar.dma_start(out=w_s[:, ko, :], in_=w[:, ko, :])
            wo_s = const.tile([128, KO, HID], BF)
            maskt_s = const.tile([CHUNK, CHUNK], BF)
            nc.sync.dma_start(out=maskt_s, in_=maskt)
            ident = const.tile([128, 128], BF)
            make_identity(nc, ident)
            identf = const.tile([128, 128], FP32)
            make_identity(nc, identf)
            eps_t = const.tile([128, 1], FP32)
            nc.vector.memset(eps_t, EPS)

            # accumulators living across the whole chunk loop
            ssm = persist.tile([128, RCHUNK], FP32)
            rmsm = persist.tile([128, RCHUNK], FP32)
            rstd_m = persist.tile([128, RCHUNK], FP32)
            ss_all = persist.tile([128, NCHUNK], FP32)     # partial sumsq
            s_sb = persist.tile([128, HPC, D], BF)         # state for o_inter
            nc.vector.memset(s_sb, 0.0)
            aiv = a2a_in  # [q][lc, f, c] transposed payload

            # ---- phase A/B/C: projections + attention, per 128-token chunk ----
            with (
                tc.tile_pool(name="xt_p", bufs=3) as xt_p,
                tc.tile_pool(name="trig", bufs=2) as trig,
                tc.tile_pool(name="work", bufs=2) as work,
                tc.tile_pool(name="small", bufs=4) as small,
                tc.tile_pool(name="pq", bufs=2, space="PSUM") as pq,
                tc.tile_pool(name="ptr", bufs=1, space="PSUM") as ptr,
                tc.tile_pool(name="po", bufs=1, space="PSUM") as po,
                tc.tile_pool(name="ps", bufs=1, space="PSUM") as ps,
            ):
                # one PSUM bank per head: a start=True from another group
                # sharing the bank would clear this group's has_written bits
                # and break the running accumulation
                s_psum = [ps.tile([128, D], FP32, tag=f"s{h}",
                                  name=f"s_psum{h}")
                          for h in range(HPC)]

                for n in range(NCHUNK):
                    # --- projection: qkvg chunk = x_chunk @ [Wq|Wk|Wv|Wg] ---
                    if n == 0:
                        xt_s = pre_xt
                    else:
                        xt_s = xt_p.tile([128, KO, CHUNK], BF)
                        xt_dma = nc.sync.dma_start(out=xt_s, in_=xt[n])
                        if n == 40:
                            xt40_dma = xt_dma
                        if n == NCHUNK - 1:
                            xt_last_dma = xt_dma
                    psum_qkvg = pq.tile([128, 4 * CPC], FP32)
                    for ko in range(KO):
                        for sl in range(2):
                            nc.tensor.matmul(
                                psum_qkvg[:, sl * 512:(sl + 1) * 512],
                                lhsT=xt_s[:, ko, :],
                                rhs=w_s[:, ko, sl * 512:(sl + 1) * 512],
                                start=(ko == 0),
                                stop=(ko == KO - 1),
                            )

                    # --- sigmoid of everything; silu(qkv) = qkv * sig ---
                    sig_f = work.tile([128, 4 * CPC], FP32, tag="sigf")
                    nc.scalar.activation(
                        out=sig_f, in_=psum_qkvg,
                        func=mybir.ActivationFunctionType.Sigmoid,
                    )
                    qkv_s = work.tile([128, 3 * CPC], FP32, tag="qkv")
                    nc.vector.tensor_mul(qkv_s, psum_qkvg[:, :3 * CPC],
                                         sig_f[:, :3 * CPC])

                    # --- per-head RMSNorm of q and k (over D=128) ---
                    sq_s = work.tile([128, 2 * CPC], FP32, tag="sq")
                    nc.vector.tensor_mul(sq_s, qkv_s[:, :2 * CPC],
                                         qkv_s[:, :2 * CPC])
                    ssum = small.tile([128, 4], FP32, tag="ssum")
                    nc.vector.reduce_sum(
                        out=ssum,
                        in_=sq_s.rearrange("p (g d) -> p g d", d=D),
                        axis=mybir.AxisListType.X,
                    )
                    rms = small.tile([128, 4], FP32, tag="rms")
                    nc.scalar.activation(
                        out=rms, in_=ssum,
                        func=mybir.ActivationFunctionType.Sqrt,
                        bias=eps_t, scale=1.0 / D,
                    )
                    rstd = small.tile([128, 4], FP32, tag="rstd")
                    nc.vector.reciprocal(out=rstd, in_=rms)
                    for g in range(4):
                        nc.vector.tensor_scalar_mul(
                            out=qkv_s[:, g * D:(g + 1) * D],
                            in0=qkv_s[:, g * D:(g + 1) * D],
                            scalar1=rstd[:, g:g + 1],
                        )

                    # --- rope on q and k (ln weights folded into tables:
                    # tab = [ln1*cos | ln2*sin | ln2*cos | ln1*sin]) ---
                    if n == 0:
                        qtab_t, ktab_t = pre_qtab, pre_ktab
                    else:
                        qtab_t = trig.tile([CHUNK, 4 * D], BF, tag="qtab")
                        nc.sync.dma_start(out=qtab_t, in_=qtab[n])
                        ktab_t = trig.tile([CHUNK, 4 * D], BF, tag="ktab")
                        nc.sync.dma_start(out=ktab_t, in_=ktab[n])

                    q_ro = work.tile([128, HPC, D], BF, tag="q_ro")
                    k_ro = work.tile([128, HPC, D], BF, tag="k_ro")
                    v_s = work.tile([128, HPC, D], BF, tag="v_s")
                    nc.vector.tensor_copy(v_s.rearrange("p h d -> p (h d)"),
                                          qkv_s[:, 2 * CPC:])
                    for src_off, tab, dst in ((0, qtab_t, q_ro),
                                              (CPC, ktab_t, k_ro)):
                        xv = qkv_s[:, src_off:src_off + CPC].rearrange(
                            "p (h d) -> p h d", d=D)
                        x1 = xv[:, :, :HALF]
                        x2 = xv[:, :, HALF:]
                        tv = tab.rearrange("p (i h f) -> p i h f", i=4, f=HALF)
                        t1 = work.tile([128, HPC, HALF], FP32, tag="ropet1")
                        t2 = work.tile([128, HPC, HALF], FP32, tag="ropet2")
                        nc.vector.tensor_mul(t1, x1, tv[:, 0])
                        nc.vector.tensor_mul(t2, x2, tv[:, 1])
                        nc.vector.tensor_sub(dst[:, :, :HALF], t1, t2)
                        nc.vector.tensor_mul(t1, x2, tv[:, 2])
                        nc.vector.tensor_mul(t2, x1, tv[:, 3])
                        nc.vector.tensor_add(dst[:, :, HALF:], t1, t2)

                    # --- transposes: qT, kT (per head) via PE ---
                    qT = work.tile([128, HPC, D], BF, tag="qT")
                    kT = work.tile([128, HPC, D], BF, tag="kT")
                    for ro, tr in ((q_ro, qT), (k_ro, kT)):
                        for h in range(HPC):
                            tp = ptr.tile([128, 128], BF, tag="tr")
                            nc.tensor.transpose(tp, ro[:, h, :], ident)
                            nc.vector.tensor_copy(tr[:, h, :], tp)

                    # --- attention ---
                    psum_o = po.tile([128, CPC], FP32)
                    prev_last_mm = None
                    for h in range(HPC):
                        sc_p = ptr.tile([128, 128], FP32, tag="tr")
                        nc.tensor.matmul(sc_p, lhsT=kT[:, h, :],
                                         rhs=qT[:, h, :], start=True, stop=True)
                        sc_s = work.tile([CHUNK, CHUNK], BF, tag="sc")
                        nc.vector.tensor_mul(sc_s, sc_p, maskt_s)
                        mm_intra = nc.tensor.matmul(
                            psum_o[:, h * D:(h + 1) * D],
                            lhsT=sc_s, rhs=v_s[:, h, :],
                            start=True, stop=(n == 0))
                        if prev_last_mm is not None:
                            # psum_o's bank is shared by both heads' groups;
                            # this start=True clears the whole bank's
                            # has_written bits, so it must not land between
                            # the other head's accumulating matmuls
                            add_dep_helper(mm_intra.ins, prev_last_mm.ins,
                                           reason="shared o-bank group order")
                        last_mm = mm_intra
                        if n > 0:
                            last_mm = nc.tensor.matmul(
                                psum_o[:, h * D:(h + 1) * D],
                                lhsT=qT[:, h, :], rhs=s_sb[:, h, :],
                                start=False, stop=True)
                        prev_last_mm = last_mm
                        if n < NCHUNK - 1:
                            nc.tensor.matmul(s_psum[h],
                                             lhsT=k_ro[:, h, :],
                                             rhs=v_s[:, h, :],
                                             start=(n == 0),
                                             stop=(n == NCHUNK - 2))
                            nc.vector.tensor_copy(s_sb[:, h, :], s_psum[h])

                    # --- g2 = o * (gnw * sig) -> a2a_in (rstd applied later,
                    # it commutes with the out-projection row-wise) ---
                    g2_t = work.tile([128, CPC], BF, tag="g2")
                    nc.vector.tensor_mul(g2_t, psum_o, sig_f[:, 3 * CPC:])
                    # ship the A2A payload pre-transposed so phase D needs
                    # only plain (non-XBAR) DMA loads
                    for h in range(HPC):
                        tp2 = ptr.tile([128, 128], BF, tag="tr",
                                       name=f"g2tp{n}_{h}")
                        nc.tensor.transpose(tp2, g2_t[:, h * D:(h + 1) * D],
                                            ident)
                        g2T = work.tile([128, CHUNK], BF, tag="g2T")
                        nc.vector.tensor_copy(g2T, tp2)
                        nc.scalar.dma_start(
                            out=aiv[n // 16][:][n % 16,
                                               h * D:(h + 1) * D, :],
                            in_=g2T)
                    # partial sumsq of o (f32, straight from psum)
                    osq = work.tile([128, CPC], FP32, tag="osq")
                    nc.scalar.activation(
                        out=osq, in_=psum_o,
                        func=mybir.ActivationFunctionType.Square,
                        accum_out=ss_all[:, n:n + 1],
                    )
                    if DEBUG:
                        do_ = work.tile([128, CPC], FP32, tag="dbgo")
                        nc.vector.tensor_copy(do_, psum_o)
                        nc.scalar.dma_start(out=dbg_o[n], in_=do_)

                    if 40 <= n < 40 + KO:
                        nc.sync.dma_start(out=wo_s[:, n - 40, :],
                                          in_=wo[:, n - 40, :])
                    if n % 16 == 15:
                        # quarter q complete: token-major ss slice, then
                        # A2A of the quarter's gated activations + RS of
                        # its sumsq -- the first three quarters' collectives
                        # overlap the remaining chunk loop
                        q = n // 16
                        ssT_p = ptr.tile([16, 128], FP32, tag="tr",
                                         name=f"ssT_p{q}")
                        nc.tensor.transpose(
                            ssT_p, ss_all[:, q * 16:(q + 1) * 16], identf)
                        ssT = small.tile([16, 128], FP32, tag="ssT",
                                         name=f"ssT{q}")
                        nc.vector.tensor_copy(ssT, ssT_p)
                        nc.scalar.dma_start(
                            out=ss_part[q][:].rearrange(
                                "(n c) -> n c", c=CHUNK),
                            in_=ssT)
                        rs_inst = nc.gpsimd.collective_compute(
                            "ReduceScatter",
                            mybir.AluOpType.add,
                            replica_groups=[list(range(N_CORES))],
                            ins=[ss_part[q][:].opt()],
                            outs=[ss_mine[q][:].opt()],
                        )
                        a2a_inst = nc.gpsimd.collective_compute(
                            "AllToAll",
                            mybir.AluOpType.bypass,
                            replica_groups=[list(range(N_CORES))],
                            ins=[a2a_in[q][:].opt()],
                            outs=[a2a_out[q][:].opt()],
                        )
                        add_dep_helper(a2a_inst.ins, rs_inst.ins, sync=False,
                                       reason="RS (tiny) ahead of A2A")
                        nc.gpsimd.dma_start(
                            out=ssm[:, q * 2:(q + 1) * 2],
                            in_=ss_mine[q][:].rearrange(
                                "(r c) -> c r", c=CHUNK))
            if DEBUG:
                nc.scalar.dma_start(out=dbg_ss, in_=ss_all)


            def emit_rstd(q):
                nc.scalar.activation(
                    out=rmsm[:, q * 2:(q + 1) * 2],
                    in_=ssm[:, q * 2:(q + 1) * 2],
                    func=mybir.ActivationFunctionType.Sqrt,
                    bias=eps_t, scale=1.0 / INNER,
                )
                nc.vector.reciprocal(
                    out=rstd_m[:, q * 2:(q + 1) * 2],
                    in_=rmsm[:, q * 2:(q + 1) * 2])

            for q in range(3):
                emit_rstd(q)


            # ---- phase D: local out-projection of my token rows ----
            # a2a_out[q][j, lh, f, c] = gated^T block: cols j*CPC+f of
            # token q*2048 + me*256 + lh*128 + c
            aov = [a2a_out[q][:].rearrange("(j l) f c -> j l f c", j=N_CORES)
                   for q in range(4)]
            with (
                tc.tile_pool(name="dwork", bufs=2) as dwork,
                tc.tile_pool(name="gl", bufs=16) as gl,
                tc.tile_pool(name="pout", bufs=2, space="PSUM") as pout,
            ):
                for r in range(RCHUNK):
                    q, lh = r // 2, r % 2
                    if r == 6:
                        emit_rstd(3)
                    out_psum = pout.tile([128, HID], FP32)
                    for ko in range(KO):
                        j, half = ko // 2, ko % 2
                        glt = gl.tile([128, CHUNK], BF, tag="glt")
                        gl_dma = nc.sync.dma_start(
                            out=glt,
                            in_=aov[q][j, lh, half * 128:(half + 1) * 128, :],
                        )
                        if r == 0 and ko == 0:
                            # keep the transposed loads (serialized against
                            # collectives) out of the chunk loop's sync stream
                            add_dep_helper(gl_dma.ins, xt_last_dma.ins,
                                           sync=False,
                                           reason="gl loads after xt stream")
                        for sl in range(HID // 512):
                            nc.tensor.matmul(
                                out_psum[:, sl * 512:(sl + 1) * 512],
                                lhsT=glt,
                                rhs=wo_s[:, ko, sl * 512:(sl + 1) * 512],
                                start=(ko == 0),
                                stop=(ko == KO - 1),
                            )
                    out_sb = dwork.tile([128, HID], FP32, tag="out_sb")
                    nc.vector.tensor_scalar_mul(
                        out=out_sb[:, :HID // 2],
                        in0=out_psum[:, :HID // 2],
                        scalar1=rstd_m[:, r:r + 1])
                    nc.scalar.activation(
                        out=out_sb[:, HID // 2:], in_=out_psum[:, HID // 2:],
                        func=mybir.ActivationFunctionType.Copy,
                        scale=rstd_m[:, r:r + 1])
                    nc.scalar.dma_start(
                        out=out[r * CHUNK:(r + 1) * CHUNK, :], in_=out_sb)

    nc.compile()
    return nc


_PROGRAM = None


def _get_program():
    global _PROGRAM
    if _PROGRAM is None:
        _PROGRAM = _build_program()
    return _PROGRAM


def _prep_inputs(hidden_states, positions, Wqkv, q_ln_w, k_ln_w, Wg, g_norm_w,
                 Wo):
    x = np.asarray(hidden_states, dtype=np.float32)
    pos = np.asarray(positions).astype(np.float32)

    # x transposed+tiled: xt[n, p, ko, t] = x[n*128+t, ko*128+p]
    xt = np.ascontiguousarray(
        x.reshape(NCHUNK, CHUNK, HID // 128, 128).transpose(0, 3, 2, 1)
    ).astype(BF16)

    # rope tables with the per-head ln weights (and q's 1/sqrt(D) scale)
    # folded in: tab = [ln1*cos | ln2*sin | ln2*cos | ln1*sin], head-dup'd
    inv_freq = (1.0 / (ROPE_BASE ** (np.arange(HALF, dtype=np.float32) / HALF))
                ).astype(np.float32)
    ang = pos[:, None] * inv_freq[None, :]              # [T, HALF]
    cos = np.cos(ang).astype(np.float32)
    sin = np.sin(ang).astype(np.float32)

    def rope_tab(ln, scale):
        l1 = (np.asarray(ln, np.float32)[:HALF] * scale)[None, :]
        l2 = (np.asarray(ln, np.float32)[HALF:] * scale)[None, :]
        tab = np.stack([l1 * cos, l2 * sin, l2 * cos, l1 * sin], axis=1)
        tab = np.tile(tab[:, :, None, :], (1, 1, 2, 1))   # dup per head
        return tab.reshape(NCHUNK, CHUNK, 4 * D).astype(BF16)

    qtab = rope_tab(q_ln_w, SCALE)
    ktab = rope_tab(k_ln_w, 1.0)

    maskt = np.triu(np.ones((CHUNK, CHUNK), dtype=np.float32)).astype(BF16)

    Wqkv = np.asarray(Wqkv, np.float32)
    Wg_ = np.asarray(Wg, np.float32)
    Wo_ = np.asarray(Wo, np.float32)
    gn = np.asarray(g_norm_w, np.float32)

    # full Wo with g_norm_w folded into its rows, [p, ko, n] layout
    Wo_g = Wo_ * gn[:, None]
    wo_r = np.ascontiguousarray(
        Wo_g.reshape(HID // 128, 128, HID).transpose(1, 0, 2)
    ).astype(BF16)

    in_maps = []
    for c in range(N_CORES):
        lo, hi = c * CPC, (c + 1) * CPC
        wc = np.concatenate(
            [Wqkv[:, lo:hi], Wqkv[:, INNER + lo:INNER + hi],
             Wqkv[:, 2 * INNER + lo:2 * INNER + hi], Wg_[:, lo:hi]], axis=1)
        w_r = np.ascontiguousarray(
            wc.reshape(HID // 128, 128, 4 * CPC).transpose(1, 0, 2)
        ).astype(BF16)
        in_maps.append({
            "xt": xt, "w": w_r, "wo": wo_r, "qtab": qtab, "ktab": ktab,
            "maskt": maskt,
        })
    return in_maps


def run(trace=False, **inputs):
    nc = _get_program()
    in_maps = _prep_inputs(**inputs)
    res = run_bass_kernel_spmd(nc, in_maps, list(range(N_CORES)), trace=trace)
    # core c's row i = (2q+lh)*128 + c128  <->  token q*2048 + c*256
    #                                               + lh*128 + c128
    stacked = np.stack([res.results[c]["out"] for c in range(N_CORES)])
    full = (stacked.reshape(N_CORES, 4, 2, CHUNK, HID)
            .transpose(1, 0, 2, 3, 4).reshape(T, HID))
    return full, res


def kernel(**inputs):
    full, _ = run(trace=False, **inputs)
    return full
